# Initial kernel scaffold
#
"""Trainium2 Bass kernel for nn_LIMADNN2_42013370090068 (dense_mlp).

Reference semantics: out depends only on x[:, 0, :] — the `state.add(...)`
neighbor loop in the torch module is not in-place, so the 65-neighbor
dimension is dead. force_prev = x[:, 0, 6:9] is a pure slice.

  q   = x[:, 0, :]                 # [B, 12]
  h   = relu(q @ W1 + b1)          # [B, 16]
  blk = relu(h @ W2 + b2)          # [B, 8]
  out = (blk @ Ws + bs) @ Wo + bo  # [B, 3]   (no relu between -> folded)

Device strategy (pure data parallel, 8 cores, batch-sharded):
  * Host slices q (12.6 MB of the 818 MB input), computes force_prev, and
    folds Ws/Wo into one [8,3] matrix (no nonlinearity between them).
  * Features-on-partitions layout: matmuls stream atoms along the PSUM
    free dimension (N=512) with lhsT = weights.
  * 8 batch-chunks packed per PE pass via block-diagonal weights:
    W1_bd [96,128], W2_bd [128,64], W3_bd [64,24]. One matmul therefore
    processes 8x512 = 4096 atoms.
  * DMA count minimized (descriptor-gen serializes): all weights+biases
    ride one packed [128, 218] DMA; inputs in 4 DMAs, outputs in 2.
  * Activations work on 1024-wide pairs of matmul outputs to amortize
    fixed per-op cost; biases fused (ScalarE relu, VectorE dual-op
    tensor_scalar). Final bias bso added on host.
"""

import numpy as np

B = 262144
F = 12
N_CORES = 8
BPC = B // N_CORES          # 32768 atoms per core
CHUNKS = 8                  # batch chunks packed on PE partitions
TILE_N = 512                # atoms per matmul column tile (fp32 PSUM bank)
SUPER = BPC // (CHUNKS * TILE_N)   # 8 supertiles per core
FREE = SUPER * TILE_N       # 4096
WCOLS = 218                 # packed weight tensor columns

# matmul operand dtype: "float32" (exact, 4 cyc/row) or "float32r"
# (1 cyc/row at N=512; reduced-precision fp32 mode)
MM_DTYPE = "float32"


def _build_nc():
    import concourse.tile as tile
    from concourse import bacc, mybir

    f32 = mybir.dt.float32
    mmdt = getattr(mybir.dt, MM_DTYPE)

    nc = bacc.Bacc("TRN2", target_bir_lowering=False, debug=False,
                   num_devices=N_CORES)

    xin = nc.dram_tensor("xin", [CHUNKS * F, FREE], f32, kind="ExternalInput")
    wpack = nc.dram_tensor("wpack", [128, WCOLS], f32, kind="ExternalInput")
    out = nc.dram_tensor("out", [24, FREE], f32, kind="ExternalOutput")

    Relu = mybir.ActivationFunctionType.Relu
    add, vmax = mybir.AluOpType.add, mybir.AluOpType.max

    def mm(ps_ap, lhsT_ap, rhs_ap):
        nc.tensor.matmul(ps_ap, lhsT_ap.bitcast(mmdt), rhs_ap.bitcast(mmdt),
                         start=True, stop=True)

    with tile.TileContext(nc) as tc:
        with (
            tc.tile_pool(name="const", bufs=1) as cpool,
            tc.tile_pool(name="xt", bufs=2) as xpool,
            tc.tile_pool(name="h", bufs=2) as hpool,
            tc.tile_pool(name="blk", bufs=2) as bpool,
            tc.tile_pool(name="osb", bufs=2) as opool,
            tc.tile_pool(name="ps1", bufs=2, space="PSUM") as ps1pool,
            tc.tile_pool(name="ps2", bufs=1, space="PSUM") as ps2pool,
            tc.tile_pool(name="ps3", bufs=1, space="PSUM") as ps3pool,
        ):
            wsb = cpool.tile([128, WCOLS], f32)
            nc.sync.dma_start(wsb[:], wpack[:])
            w1_ap = wsb[0:96, 0:128]
            w2_ap = wsb[0:128, 128:192]
            w3_ap = wsb[0:64, 192:216]
            b1_ap = wsb[0:128, 216:217]
            b2_ap = wsb[0:64, 217:218]

            osb = None
            for p in range(SUPER // 2):
                cols = slice(2 * p * TILE_N, 2 * (p + 1) * TILE_N)
                xt = xpool.tile([96, 2 * TILE_N], f32)
                nc.sync.dma_start(xt[:], xin[:, cols])

                ps1 = ps1pool.tile([128, 2 * TILE_N], f32)
                mm(ps1[:, 0:TILE_N], w1_ap, xt[:, 0:TILE_N])
                mm(ps1[:, TILE_N:], w1_ap, xt[:, TILE_N:])
                h = hpool.tile([128, 2 * TILE_N], f32)
                nc.scalar.activation(h[:], ps1[:], Relu, bias=b1_ap)

                ps2 = ps2pool.tile([64, 2 * TILE_N], f32)
                mm(ps2[:, 0:TILE_N], w2_ap, h[:, 0:TILE_N])
                mm(ps2[:, TILE_N:], w2_ap, h[:, TILE_N:])
                blk = bpool.tile([64, 2 * TILE_N], f32)
                nc.vector.tensor_scalar(blk[:], ps2[:], b2_ap, 0.0, add, vmax)

                ps3 = ps3pool.tile([24, 2 * TILE_N], f32)
                mm(ps3[:, 0:TILE_N], w3_ap, blk[:, 0:TILE_N])
                mm(ps3[:, TILE_N:], w3_ap, blk[:, TILE_N:])

                if p % 2 == 0:
                    osb = opool.tile([24, 4 * TILE_N], f32)
                    nc.scalar.copy(osb[:, 0:2 * TILE_N], ps3[:])
                else:
                    nc.vector.tensor_copy(osb[:, 2 * TILE_N:], ps3[:])
                    nc.sync.dma_start(
                        out[:, (p - 1) * 2 * TILE_N:(p + 1) * 2 * TILE_N],
                        osb[:])

    nc.finalize()
    return nc


def _host_prep(x, W1, b1, W2, b2, Ws, bs, Wo, bo):
    x = np.asarray(x)
    W1 = np.asarray(W1, dtype=np.float32)
    b1 = np.asarray(b1, dtype=np.float32)
    W2 = np.asarray(W2, dtype=np.float32)
    b2 = np.asarray(b2, dtype=np.float32)
    Ws = np.asarray(Ws, dtype=np.float32)
    bs = np.asarray(bs, dtype=np.float32)
    Wo = np.asarray(Wo, dtype=np.float32)
    bo = np.asarray(bo, dtype=np.float32)

    q = np.ascontiguousarray(x[:, 0, :], dtype=np.float32)       # [B, 12]
    force_prev = np.ascontiguousarray(x[:, 0, 6:9], dtype=np.float32)

    # Fold the two linear layers that have no nonlinearity between them.
    Wso = (Ws.astype(np.float64) @ Wo.astype(np.float64)).astype(np.float32)
    bso = (bs.astype(np.float64) @ Wo.astype(np.float64)
           + bo.astype(np.float64)).astype(np.float32)

    wpack = np.zeros((128, WCOLS), np.float32)
    for c in range(CHUNKS):
        wpack[c * 12:(c + 1) * 12, c * 16 + 0:(c + 1) * 16] = W1
        wpack[c * 16:(c + 1) * 16, 128 + c * 8:128 + (c + 1) * 8] = W2
        wpack[c * 8:(c + 1) * 8, 192 + c * 3:192 + (c + 1) * 3] = Wso
        wpack[c * 16:(c + 1) * 16, 216] = b1
        wpack[c * 8:(c + 1) * 8, 217] = b2

    in_maps = []
    for c in range(N_CORES):
        qc = q[c * BPC:(c + 1) * BPC]
        # atom n = t*4096 + ch*512 + a  ->  partition 12*ch+f, free t*512+a
        Ac = np.ascontiguousarray(
            qc.reshape(SUPER, CHUNKS, TILE_N, F)
              .transpose(1, 3, 0, 2).reshape(CHUNKS * F, FREE))
        in_maps.append({"xin": Ac, "wpack": wpack})
    return in_maps, force_prev, bso


def _host_gather(results, bso):
    out = np.empty((B, 3), np.float32)
    for c in range(N_CORES):
        Oc = results[c]["out"]                                   # [24, 4096]
        oc = (Oc.reshape(CHUNKS, 3, SUPER, TILE_N)
                .transpose(2, 0, 3, 1).reshape(BPC, 3))
        out[c * BPC:(c + 1) * BPC] = oc + bso
    return out


def kernel(x, W1, b1, W2, b2, Ws, bs, Wo, bo):
    from concourse.bass_utils import run_bass_kernel_spmd

    in_maps, force_prev, bso = _host_prep(x, W1, b1, W2, b2, Ws, bs, Wo, bo)
    nc = _build_nc()
    res = run_bass_kernel_spmd(nc, in_maps, core_ids=list(range(N_CORES)))
    out = _host_gather(res.results, bso)
    return (out, force_prev)



# revision 11
# speedup vs baseline: 1.0070x; 1.0070x over previous
"""Trainium2 Bass kernel for nn_LIMADNN2_42013370090068 (dense_mlp).

Reference semantics: out depends only on x[:, 0, :] — the `state.add(...)`
neighbor loop in the torch module is not in-place, so the 65-neighbor
dimension is dead. force_prev = x[:, 0, 6:9] is a pure slice.

  q   = x[:, 0, :]                 # [B, 12]
  h   = relu(q @ W1 + b1)          # [B, 16]
  blk = relu(h @ W2 + b2)          # [B, 8]
  out = (blk @ Ws + bs) @ Wo + bo  # [B, 3]   (no relu between -> folded)

Device strategy (pure data parallel, 8 cores, batch-sharded):
  * Host slices q (12.6 MB of the 818 MB input), computes force_prev, and
    folds Ws/Wo into one [8,3] matrix (no nonlinearity between them).
  * All matmul operands bf16 (fp32 PSUM accumulate): 1 cyc/col on the PE
    vs 4 for fp32, and half the HBM traffic. Measured end-to-end rel err
    ~7e-3 vs the 2e-2 gate.
  * Features-on-partitions; atoms stream along the free dim. 8
    batch-chunks packed per matmul via block-diagonal weights.
  * Layer 2 packs TWO 512-col groups into one [128,512] PSUM tile
    (partition-offset outputs), halving relu2 + L3 columns; layer 3 uses
    16-chunk block-diag [128,48] and two supergroups share a [96,512]
    PSUM tile so one copy+bias (DVE) and one DMA move 16384 atoms out.
  * PE warm-up: dummy matmuls on the weight tile run while input DMAs
    stream, so the HAM un-throttles (1.2 -> 2.4 GHz) before real work.
  * Biases ride the PSUM->SBUF ops (ACT relu bias, DVE tensor_scalar);
    the folded output bias bso is added in the final copy on-device.
"""

import numpy as np
import ml_dtypes

B = 262144
F = 12
N_CORES = 8
BPC = B // N_CORES          # 32768 atoms per core
CHUNKS = 8                  # batch chunks packed on PE partitions (L1)
TILE_N = 512                # atoms per matmul column tile (one PSUM bank)
GROUPS = 8                  # 512-col groups per core (4096 atoms each)
SUPER = 4                   # supergroups = pairs of groups
FREE = GROUPS * TILE_N      # 4096 input columns per core
WCOLS = 246                 # packed weight tensor columns (bf16; biases are
                            # f32 bit-packed into bf16 pairs at cols 240-245)
N_WARMUP = 8                # PE warm-up matmuls (N=240 each)

BF16 = ml_dtypes.bfloat16


def _build_nc():
    import concourse.tile as tile
    from concourse import bacc, mybir

    f32 = mybir.dt.float32
    bf16 = mybir.dt.bfloat16

    nc = bacc.Bacc("TRN2", target_bir_lowering=False, debug=False,
                   num_devices=N_CORES)

    xin = nc.dram_tensor("xin", [CHUNKS * F, FREE], bf16, kind="ExternalInput")
    wpack = nc.dram_tensor("wpack", [128, WCOLS], bf16, kind="ExternalInput")
    # 112 rows: supergroup-pair halves at partition offsets 0 and 64 (matmul
    # output base partition must be 0/32/64); rows 48-63 are dead padding.
    out = nc.dram_tensor("out", [112, SUPER // 2 * TILE_N], bf16,
                         kind="ExternalOutput")

    Relu = mybir.ActivationFunctionType.Relu
    add, vmax = mybir.AluOpType.add, mybir.AluOpType.max

    def mm(ps_ap, lhsT_ap, rhs_ap):
        nc.tensor.matmul(ps_ap, lhsT_ap, rhs_ap, start=True, stop=True)

    with tile.TileContext(nc) as tc:
        with (
            tc.tile_pool(name="const", bufs=1) as cpool,
            tc.tile_pool(name="xt", bufs=2) as xpool,
            tc.tile_pool(name="h", bufs=2) as hpool,
            tc.tile_pool(name="blk", bufs=2) as bpool,
            tc.tile_pool(name="osb", bufs=2) as opool,
            tc.tile_pool(name="psw", bufs=1, space="PSUM") as pswpool,
            tc.tile_pool(name="ps1", bufs=2, space="PSUM") as ps1pool,
            tc.tile_pool(name="ps2", bufs=2, space="PSUM") as ps2pool,
            tc.tile_pool(name="ps3", bufs=1, space="PSUM") as ps3pool,
        ):
            wsb = cpool.tile([128, WCOLS], bf16)
            nc.sync.dma_start(wsb[:], wpack[:])
            w1 = wsb[0:96, 0:128]
            w2 = wsb[0:128, 128:192]
            w3 = wsb[0:128, 192:240]
            b1 = wsb[0:128, 240:242].bitcast(f32)
            b2 = wsb[0:128, 242:244].bitcast(f32)
            bso = wsb[0:112, 244:246].bitcast(f32)

            # Keep the PE busy (HAM warm-up) while input chunks stream in.
            psw = pswpool.tile([128, 240], f32)
            for _ in range(N_WARMUP):
                mm(psw[:], wsb[0:128, 0:128], wsb[0:128, 0:240])

            ps3 = osb = None
            for s in range(SUPER):
                cols = slice(2 * TILE_N * s, 2 * TILE_N * (s + 1))
                xt = xpool.tile([CHUNKS * F, 2 * TILE_N], bf16)
                nc.sync.dma_start(xt[:], xin[:, cols])

                ps1 = ps1pool.tile([128, 2 * TILE_N], f32)
                mm(ps1[:, 0:TILE_N], w1, xt[:, 0:TILE_N])
                mm(ps1[:, TILE_N:], w1, xt[:, TILE_N:])
                h = hpool.tile([128, 2 * TILE_N], bf16)
                nc.scalar.activation(h[:], ps1[:], Relu, bias=b1)

                ps2 = ps2pool.tile([128, TILE_N], f32)
                mm(ps2[0:64, :], w2, h[:, 0:TILE_N])
                mm(ps2[64:128, :], w2, h[:, TILE_N:])
                blk = bpool.tile([128, TILE_N], bf16)
                nc.vector.tensor_scalar(blk[:], ps2[:], b2, 0.0, add, vmax)

                if s % 2 == 0:
                    ps3 = ps3pool.tile([112, TILE_N], f32)
                off = 64 * (s % 2)
                mm(ps3[off:off + 48, :], w3, blk[:])

                if s % 2 == 1:
                    osb = opool.tile([112, TILE_N], bf16)
                    nc.vector.tensor_scalar(osb[:], ps3[:], bso, None, add)
                    p = s // 2
                    nc.sync.dma_start(
                        out[:, p * TILE_N:(p + 1) * TILE_N], osb[:])

    nc.finalize()
    return nc


def _host_prep(x, W1, b1, W2, b2, Ws, bs, Wo, bo):
    x = np.asarray(x)
    W1 = np.asarray(W1, dtype=np.float32)
    b1 = np.asarray(b1, dtype=np.float32)
    W2 = np.asarray(W2, dtype=np.float32)
    b2 = np.asarray(b2, dtype=np.float32)
    Ws = np.asarray(Ws, dtype=np.float32)
    bs = np.asarray(bs, dtype=np.float32)
    Wo = np.asarray(Wo, dtype=np.float32)
    bo = np.asarray(bo, dtype=np.float32)

    q = np.ascontiguousarray(x[:, 0, :], dtype=np.float32)       # [B, 12]
    force_prev = np.ascontiguousarray(x[:, 0, 6:9], dtype=np.float32)

    # Fold the two linear layers that have no nonlinearity between them.
    Wso = (Ws.astype(np.float64) @ Wo.astype(np.float64)).astype(np.float32)
    bso = (bs.astype(np.float64) @ Wo.astype(np.float64)
           + bo.astype(np.float64)).astype(np.float32)

    wts = np.zeros((128, 240), np.float32)
    for c in range(CHUNKS):
        wts[c * 12:(c + 1) * 12, c * 16:(c + 1) * 16] = W1
        wts[c * 16:(c + 1) * 16, 128 + c * 8:128 + (c + 1) * 8] = W2
    for k in range(16):
        wts[k * 8:(k + 1) * 8, 192 + k * 3:192 + (k + 1) * 3] = Wso
    p = np.arange(128)
    biases = np.zeros((128, 3), np.float32)
    biases[:, 0] = b1[p % 16]
    biases[:, 1] = b2[p % 8]
    biases[0:48, 2] = bso[p[0:48] % 3]
    biases[64:112, 2] = bso[p[0:48] % 3]
    wpack = np.zeros((128, WCOLS), BF16)
    wpack[:, 0:240] = wts.astype(BF16)
    # f32 biases bit-packed as bf16 pairs (little-endian), bitcast on device
    wpack[:, 240:246] = biases.view(np.uint16).view(BF16)

    qb = q.astype(BF16)
    in_maps = []
    for c in range(N_CORES):
        qc = qb[c * BPC:(c + 1) * BPC]
        # atom n = g*4096 + ch*512 + a  ->  partition 12*ch+f, col g*512+a
        Ac = np.ascontiguousarray(
            qc.reshape(GROUPS, CHUNKS, TILE_N, F)
              .transpose(1, 3, 0, 2).reshape(CHUNKS * F, FREE))
        in_maps.append({"xin": Ac, "wpack": wpack})
    return in_maps, force_prev


def _host_gather(results):
    out = np.empty((B, 3), np.float32)
    for c in range(N_CORES):
        Oc = np.asarray(results[c]["out"]).astype(np.float32)    # [112, 1024]
        Oc = np.concatenate([Oc[0:48], Oc[64:112]])              # drop pad
        # row p = 48*sp + 24*gp + 3*ch + f;  col = t*512 + j
        # atom n = (4t + 2sp + gp)*4096 + ch*512 + j
        oc = (Oc.reshape(2, 2, 8, 3, 2, TILE_N)
                .transpose(4, 0, 1, 2, 5, 3).reshape(BPC, 3))
        out[c * BPC:(c + 1) * BPC] = oc
    return out


_LAST_RES = None  # BassKernelResults of the most recent run (for test harness)


def kernel(x, W1, b1, W2, b2, Ws, bs, Wo, bo):
    global _LAST_RES
    from concourse.bass_utils import run_bass_kernel_spmd

    in_maps, force_prev = _host_prep(x, W1, b1, W2, b2, Ws, bs, Wo, bo)
    nc = _build_nc()
    res = run_bass_kernel_spmd(nc, in_maps, core_ids=list(range(N_CORES)))
    _LAST_RES = res
    out = _host_gather(res.results)
    return (out, force_prev)
